# revision 1
# baseline (speedup 1.0000x reference)
"""BiasedSelfAttention Trainium2 kernel, 8-core SPMD.

Reference computation (per batch b, head h):
    qkv = x @ W_attn + b_attn;  Q,K,V = split(qkv)
    S   = Q K^T / sqrt(hd)
    A   = softmax(S, axis=-1) + attn_B          (post-softmax additive bias)
    y   = A @ V

Sharding: 2 batches x 16 heads = 32 (b,h) pairs -> 4 heads/core,
core c handles batch c//4, heads [4*(c%4), 4*(c%4)+4).

Per-core kernel (natural [sq, sk] orientation):
  - QKV projection from host-pretransposed x^T with fp32r matmuls
    (Q^T/K^T packed [128, s] per head; V [s, 64] bf16, bias via K=1 matmul)
  - S tiles on PE (fp32r, N=512), exp+rowsum fused on ACT (accum_out)
  - A = expS * (1/rowsum) + B in one DVE scalar_tensor_tensor pass (bf16 out)
  - A^T via PE transpose (bf16, 1 cyc/row), PSUM->SBUF copies split DVE/ACT
  - y = sum_k A^T_chunk.T @ V_chunk accumulated in PSUM
"""

import numpy as np
import ml_dtypes


def _to_bf16(a):
    return a.astype(ml_dtypes.bfloat16)


import concourse.bass as bass
import concourse.mybir as mybir
import concourse.tile as tile
from concourse import bacc
from concourse.bass_utils import run_bass_kernel_spmd
from concourse.masks import make_identity

B, S, D = 2, 2048, 1024
H, HD = 16, 64
NCORES = 8
HPC = 4                 # heads per core
GD = HPC * HD           # 256 per-core output columns
KO = D // 128           # 8 contraction chunks for QKV
SQ = S // 128           # 16 seq chunks of 128
ST = S // 512           # 4 seq tiles of 512

fp32 = mybir.dt.float32
fp32r = mybir.dt.float32r
bf16 = mybir.dt.bfloat16

_CACHED_NC = None


def build_nc(repeat=1, stages=4, no_tr=False, act_share=2,
             load_dve=0, load_act=0, load_pe=0, load_dma=0):
    """repeat>1 wraps the whole body in a hardware loop (for HW timing).

    stages (ablation for profiling): 4=full, 3=no PV matmuls,
    2=also no transposes, 1=also no exp/assembly (S matmuls + B DMA only),
    0=QKV + B DMA only.
    """
    nc = bacc.Bacc()

    xT = nc.declare_dram_parameter("xT", [D, S], fp32r, isOutput=False)
    # head-pair packed Q/K weights: [:, i, 0, :] = Q cols of heads (2i, 2i+1),
    # [:, i, 1, :] = K cols of heads (2i, 2i+1)
    wqk = nc.declare_dram_parameter("wqk", [D, 2, 2, 128], fp32r, isOutput=False)
    wv = nc.declare_dram_parameter("wv", [D, GD], fp32r, isOutput=False)
    bqk = nc.declare_dram_parameter("bqk", [128, 2, 2], fp32, isOutput=False)
    bv = nc.declare_dram_parameter("bv", [1, GD], bf16, isOutput=False)
    Bb = nc.declare_dram_parameter("Bb", [HPC, S, S], bf16, isOutput=False)
    y = nc.declare_dram_parameter("y", [S, GD], fp32, isOutput=True)

    import contextlib

    with tile.TileContext(nc) as tc:
        with (
            tc.For_i(0, repeat, 1) if repeat > 1 else contextlib.nullcontext(),
            tc.tile_pool(name="persist", bufs=1) as persist,
            tc.tile_pool(name="small", bufs=1) as small,
        ):
            # ---- persistent SBUF tensors ----
            # per head-pair: partitions 0:64 = head 2i, 64:128 = head 2i+1;
            # free dim: [:, 0, :] = Q^T rows, [:, 1, :] = K^T rows
            qk2 = [persist.tile([128, 2, S], fp32r, tag=f"qk2_{i}", name=f"qk2_{i}")
                   for i in range(HPC // 2)]
            v_sb = persist.tile([128, SQ, GD], bf16, tag="v_sb")
            ident = small.tile([128, 128], bf16, tag="ident")
            make_identity(nc, ident)
            ident_f32 = small.tile([64, 64], fp32, tag="ident_f32")
            make_identity(nc, ident_f32)
            bqk_sb = small.tile([128, 2, 2], fp32, tag="bqk_sb")
            nc.sync.dma_start(out=bqk_sb, in_=bqk[:, :])
            bv_sb = small.tile([1, GD], bf16, tag="bv_sb")
            nc.sync.dma_start(out=bv_sb, in_=bv[:, :])
            ones1 = small.tile([1, 128], bf16, tag="ones1")
            nc.vector.memset(ones1, 1.0)

            # scratch tiles for differential engine-loading experiments
            if load_dve or load_act or load_pe or load_dma:
                scr_a = small.tile([128, 1024], fp32, tag="scr_a")
                scr_b = small.tile([128, 1024], fp32, tag="scr_b")
                scr_c = small.tile([128, 1024], bf16, tag="scr_c")
                nc.vector.memset(scr_a, 1.0)
                scr_w = small.tile([128, 128], fp32r, tag="scr_w")
                nc.vector.memset(scr_w.bitcast(fp32), 1.0)

            def emit_load(unit_idx):
                for _ in range(load_dve):
                    nc.vector.tensor_copy(scr_b, scr_a)
                for _ in range(load_act):
                    nc.scalar.copy(scr_b, scr_a)
                for _ in range(load_pe):
                    pl = psum_load.tile([128, 512], fp32, tag="ps_load")
                    for t2 in range(4):
                        nc.tensor.matmul(pl, scr_w,
                                         scr_a.bitcast(fp32r)[:, :512],
                                         start=(t2 == 0), stop=(t2 == 3))
                for k2 in range(load_dma):
                    r0 = ((unit_idx * 3 + k2) % SQ) * 128
                    nc.sync.dma_start(
                        out=scr_c,
                        in_=Bb[unit_idx % HPC, r0:r0 + 128, :1024],
                    )

            # ---- phase 1: QKV projection ----
            with (
                tc.tile_pool(name="p1sb", bufs=1) as p1sb,
                tc.tile_pool(name="p1ps", bufs=2, space="PSUM") as p1ps,
                tc.tile_pool(name="p1psv", bufs=2, space="PSUM") as p1psv,
            ):
                xts = p1sb.tile([128, KO, S], fp32r, tag="xts")
                nc.sync.dma_start(
                    out=xts, in_=xT.rearrange("(ko p) s -> p ko s", p=128)
                )
                wqk_sb = p1sb.tile([128, KO, 2, 2, 128], fp32r, tag="wqk_sb")
                nc.sync.dma_start(
                    out=wqk_sb, in_=wqk.rearrange("(ko p) i qk m -> p ko i qk m", p=128)
                )
                wv_sb = p1sb.tile([128, KO, GD], fp32r, tag="wv_sb")
                nc.sync.dma_start(
                    out=wv_sb, in_=wv.rearrange("(ko p) n -> p ko n", p=128)
                )

                # Q^T (resp K^T) of a head pair land on partitions 0:64 / 64:128
                for i in range(HPC // 2):
                    for qk in range(2):
                        for t in range(ST):
                            ps = p1ps.tile([128, 512], fp32, tag="ps_qk")
                            for ko in range(KO):
                                nc.tensor.matmul(
                                    ps,
                                    wqk_sb[:, ko, i, qk, :],
                                    xts[:, ko, t * 512:(t + 1) * 512],
                                    start=(ko == 0),
                                    stop=(ko == KO - 1),
                                )
                            # PSUM -> SBUF + per-partition bias (rounds to fp32r)
                            nc.scalar.activation(
                                qk2[i][:, qk, t * 512:(t + 1) * 512],
                                ps,
                                mybir.ActivationFunctionType.Identity,
                                bias=bqk_sb[:, i, qk:qk + 1],
                                scale=1.0,
                            )

                # V (all heads packed on free dim): [sk, GD] in bf16
                for kc in range(SQ):
                    psv = p1psv.tile([128, GD], fp32, tag="ps_v")
                    for ko in range(KO):
                        nc.tensor.matmul(
                            psv,
                            xts[:, ko, kc * 128:(kc + 1) * 128],
                            wv_sb[:, ko, :],
                            start=(ko == 0),
                            stop=False,
                        )
                    nc.tensor.matmul(psv, ones1, bv_sb, start=False, stop=True)
                    nc.vector.tensor_copy(v_sb[:, kc, :], psv)

            # ---- phase 2: attention, 512-row superblocks ----
            NSB = SQ // 4                       # 4 superblocks of 512 rows
            yv = y.rearrange("(sb c p) n -> sb p c n", p=128, c=4)
            Bv = Bb.rearrange("h (sb c p) s -> h sb p c s", p=128, c=4)
            with (
                tc.tile_pool(name="p2sb", bufs=2) as p2sb,
                tc.tile_pool(name="p2sb3", bufs=3) as p2sb3,
                tc.tile_pool(name="bpool", bufs=2) as bpool,
                tc.tile_pool(name="ypool", bufs=2) as ypool,
                tc.tile_pool(name="ps_s", bufs=(1 if load_pe else 2), space="PSUM") as psum_s,
                tc.tile_pool(name="ps_t", bufs=2, space="PSUM") as psum_t,
                tc.tile_pool(name="ps_yt", bufs=1, space="PSUM") as psum_yt,
                tc.tile_pool(name="ps_y2", bufs=1, space="PSUM") as psum_y2,
                tc.tile_pool(name="ps_load", bufs=1, space="PSUM") as psum_load,
            ):
                for sb in range(NSB):
                    y_sb = ypool.tile([128, 4, GD], fp32, tag="y_sb")
                    if stages < 4:
                        nc.gpsimd.memset(y_sb, 0.0)
                    for h in range(HPC):
                        emit_load(sb * HPC + h)
                        # bias rows for this (head, superblock): partition p
                        # holds rows {sb*512 + c*128 + p : c in 0..3};
                        # 4 contiguous 1MB transfers (best DMA efficiency)
                        Bt = bpool.tile([128, 4, S], bf16, tag="Bt")
                        for c in range(4):
                            nc.sync.dma_start(
                                out=Bt[:, c, :], in_=Bv[h, sb, :, c, :]
                            )

                        pair, off = h // 2, 64 * (h % 2)
                        if stages < 1:
                            continue

                        # A^T chunks for the whole superblock, bf16:
                        # AT4[sk_p, kc, c, sq_col] = A[sb*512+c*128+sq_col,
                        #                              kc*128+sk_p]
                        AT4 = p2sb.tile([128, SQ, 4, 128], bf16, tag="AT4")

                        for c in range(4):
                            q16 = 4 * sb + c
                            expS = p2sb3.tile([128, S], bf16, tag="expS")
                            parts = p2sb3.tile([128, 2], fp32, tag="parts")
                            for t in range(2):
                                # two-bank PSUM halves -> pipelining
                                ps_s = psum_s.tile([128, 1024], fp32,
                                                   tag="ps_s")
                                for t2 in range(2):
                                    tt = 2 * t + t2
                                    nc.tensor.matmul(
                                        ps_s[:, t2 * 512:(t2 + 1) * 512],
                                        qk2[pair][off:off + 64, 0,
                                                  q16 * 128:(q16 + 1) * 128],
                                        qk2[pair][off:off + 64, 1,
                                                  tt * 512:(tt + 1) * 512],
                                        start=True,
                                        stop=True,
                                    )
                                if stages < 2:
                                    continue
                                # exp(S/8) with fused partial row-sums
                                nc.scalar.activation(
                                    expS[:, t * 1024:(t + 1) * 1024],
                                    ps_s,
                                    mybir.ActivationFunctionType.Exp,
                                    scale=0.125,
                                    accum_out=parts[:, t:t + 1],
                                )
                            if stages < 2:
                                continue
                            sums = p2sb.tile([128, 1], fp32, tag="sums")
                            nc.vector.reduce_sum(
                                out=sums, in_=parts,
                                axis=mybir.AxisListType.X,
                            )
                            recip = p2sb.tile([128, 1], fp32, tag="recip")
                            nc.vector.reciprocal(recip, sums)

                            # A = expS * recip + B   (bf16)
                            At = p2sb3.tile([128, S], bf16, tag="At")
                            nc.vector.scalar_tensor_tensor(
                                At,
                                expS,
                                recip,
                                Bt[:, c, :],
                                op0=mybir.AluOpType.mult,
                                op1=mybir.AluOpType.add,
                            )

                            if stages < 3:
                                continue
                            # transpose A into AT4; copies split DVE/ACT
                            for j in range(SQ // 4):
                                if no_tr:
                                    dst = AT4[:, 4 * j:4 * j + 4, c, :]
                                    src = At[:, 4 * j * 128:(4 * j + 4) * 128]
                                    if j % 2 == 0:
                                        nc.vector.tensor_copy(dst, src)
                                    else:
                                        nc.scalar.copy(dst, src)
                                    continue
                                ps_tr = psum_t.tile([128, 4, 128], bf16,
                                                    tag="ps_tr")
                                for jj in range(4):
                                    kc = 4 * j + jj
                                    nc.tensor.transpose(
                                        ps_tr[:, jj, :],
                                        At[:, kc * 128:(kc + 1) * 128],
                                        ident,
                                    )
                                dst = AT4[:, 4 * j:4 * j + 4, c, :]
                                if (j + c) % 4 < act_share:
                                    nc.scalar.copy(dst, ps_tr)
                                else:
                                    nc.vector.tensor_copy(dst, ps_tr)

                        if stages < 4:
                            continue
                        # y^T[dv, sq] over the 512-row superblock
                        yt_ps = psum_yt.tile([64, 512], fp32, tag="yt_ps")
                        for kc in range(SQ):
                            nc.tensor.matmul(
                                yt_ps,
                                v_sb[:, kc, h * HD:(h + 1) * HD],
                                AT4[:, kc, :, :],
                                start=(kc == 0),
                                stop=(kc == SQ - 1),
                            )
                        yt_sb = p2sb.tile([64, 4, 128], fp32, tag="yt_sb")
                        nc.vector.tensor_copy(yt_sb, yt_ps)
                        # transpose y^T back to [sq, dv] in 128-col blocks
                        y2_ps = psum_y2.tile([128, 4, HD], fp32, tag="y2_ps")
                        for c in range(4):
                            nc.tensor.transpose(
                                y2_ps[:, c, :], yt_sb[:, c, :], ident_f32
                            )
                        nc.vector.tensor_copy(
                            y_sb[:, :, h * HD:(h + 1) * HD], y2_ps
                        )

                    nc.sync.dma_start(out=yv[sb], in_=y_sb)

    nc.finalize()
    return nc


def _prep_core_inputs(x, attn_B, W_attn, b_attn, core):
    bi, g = core // 4, core % 4
    h0 = HPC * g
    xT = np.ascontiguousarray(x[bi].T)                       # [D, S]
    wqk = np.empty((D, 2, 2, 128), np.float32)
    bqk = np.empty((128, 2, 2), np.float32)
    for i in range(HPC // 2):
        for j in range(2):                                   # head within pair
            gh = h0 + 2 * i + j
            sl = slice(64 * j, 64 * j + 64)
            wqk[:, i, 0, sl] = W_attn[:, gh * 64:(gh + 1) * 64]
            wqk[:, i, 1, sl] = W_attn[:, D + gh * 64:D + (gh + 1) * 64]
            bqk[sl, i, 0] = b_attn[gh * 64:(gh + 1) * 64]
            bqk[sl, i, 1] = b_attn[D + gh * 64:D + (gh + 1) * 64]
    wv = np.ascontiguousarray(W_attn[:, 2 * D + g * GD:2 * D + (g + 1) * GD])
    bv = np.ascontiguousarray(b_attn[2 * D + g * GD:2 * D + (g + 1) * GD])
    Bb = np.ascontiguousarray(attn_B[bi, h0:h0 + HPC])
    return {
        "xT": xT, "wqk": wqk, "wv": wv, "bqk": bqk,
        "bv": _to_bf16(bv.reshape(1, GD)), "Bb": _to_bf16(Bb),
    }


def kernel(x, attn_B, W_attn, b_attn):
    global _CACHED_NC
    x = np.asarray(x, np.float32)
    attn_B = np.asarray(attn_B, np.float32)
    W_attn = np.asarray(W_attn, np.float32)
    b_attn = np.asarray(b_attn, np.float32)

    if _CACHED_NC is None:
        _CACHED_NC = build_nc()
    nc = _CACHED_NC

    in_maps = [
        _prep_core_inputs(x, attn_B, W_attn, b_attn, c) for c in range(NCORES)
    ]
    res = run_bass_kernel_spmd(nc, in_maps, core_ids=list(range(NCORES)))

    out = np.empty((B, S, D), np.float32)
    for c in range(NCORES):
        bi, g = c // 4, c % 4
        out[bi, :, g * GD:(g + 1) * GD] = res.results[c]["y"]
    return out



# revision 27
# speedup vs baseline: 1.4157x; 1.4157x over previous
"""BiasedSelfAttention Trainium2 kernel, 8-core SPMD — transposed fp8 scheme.

Reference computation (per batch b, head h):
    qkv = x @ W_attn + b_attn;  Q,K,V = split(qkv)
    S   = Q K^T / sqrt(hd)
    A   = softmax(S, axis=-1) + attn_B          (post-softmax additive bias)
    y   = A @ V

Sharding: 2 batches x 16 heads = 32 (b,h) pairs -> 4 heads/core,
core c handles batch c//4, heads [4*(c%4), 4*(c%4)+4).

Per-core kernel (TRANSPOSED orientation — no A assembly, no A^T transposes):
  - QKV projection in bf16 (Q^T/K^T per head-pair packed on partitions)
  - S^T = K Q^T computed directly on PE (bf16 in, f32 PSUM)
  - exp(S^T/8 - 3) on ACT -> expST fp8e4 (shift keeps max ~e^2.8 << 240;
    softmax shift-invariance makes it exact since denominators use the
    same quantized values)
  - U'^T = V'^T expST via fp8 DoubleRow matmuls (2 sk-chunks/instr, 0.5
    cyc/row): V' = [8*V, 8] ones-augmented -> row 64 = 8*softmax-denoms FREE.
    fp8 is safe here: the whole softmax term is ~25x smaller than the BV term.
  - BV^T = V^T B^T in bf16 (fp8 would put ~6% on the DOMINANT y component:
    error and signal both grow as sqrt(N) in the sum, no averaging-down)
  - y chunk = transpose(U'^T)*recip(denom) + transpose(BV^T); y stored bf16,
    cast to f32 on host
  - B^T host-prepped fp8 in DMA-friendly layout (8KB contiguous per
    partition -> 128-descriptor DMAs)
  - software pipelining at tile granularity: S^T(h) chunk production
    interleaved with U/BV(h-1) work units in the PE FIFO; S^T(0)
    interleaved with the QKV projection itself.
"""

import numpy as np
import ml_dtypes


def _to_bf16(a):
    return a.astype(ml_dtypes.bfloat16)


def _to_fp8(a):
    return a.astype(ml_dtypes.float8_e4m3)


import concourse.bass as bass
import concourse.mybir as mybir
import concourse.tile as tile
from concourse import bacc
from concourse.bass_utils import run_bass_kernel_spmd
from concourse.masks import make_identity

B, S, D = 2, 2048, 1024
H, HD = 16, 64
NCORES = 8
HPC = 4                 # heads per core
GD = HPC * HD           # 256 per-core output columns
KO = D // 128           # 8 contraction chunks for QKV
SQ = S // 128           # 16 seq chunks of 128
ST = S // 512           # 4 seq tiles of 512
KC2 = SQ // 2           # 8 double-chunks for fp8 DoubleRow

fp32 = mybir.dt.float32
fp32r = mybir.dt.float32r
bf16 = mybir.dt.bfloat16
fp8 = mybir.dt.float8e4

EXP_SHIFT = -3.0        # exp(S/8 - 3): keeps fp8 range safe; cancels in softmax
BSCALE = 64.0           # host-side scale on B^T for fp8 resolution
VSCALE = 8.0            # device-side scale on V for fp8 resolution

_CACHED_NC = None


def build_nc(repeat=1):
    """repeat>1 wraps the whole body in a hardware loop (for HW timing)."""
    nc = bacc.Bacc()

    xT = nc.declare_dram_parameter("xT", [D, S], bf16, isOutput=False)
    # all QKV weights packed per-partition-contiguous:
    # wall[p, ko, i*256+qk*128+m] = W col m of head-pair i Q/K (d = ko*128+p),
    # wall[p, ko, 512+n] = V weight col n
    wall = nc.declare_dram_parameter("wall", [128, KO, 768], bf16,
                                     isOutput=False)
    bqk = nc.declare_dram_parameter("bqk", [128, 2, 2], fp32, isOutput=False)
    bv = nc.declare_dram_parameter("bv", [1, GD], bf16, isOutput=False)
    # host-prepped transposed bias, bf16, quarter-major:
    # BT2[h, q, p, kc, c] = attn_B[bi, h0+h, q*512+c, kc*128+p]
    BT2 = nc.declare_dram_parameter("BT2", [HPC, 4, 128, SQ, 512], bf16,
                                    isOutput=False)
    y = nc.declare_dram_parameter("y", [S, GD], bf16, isOutput=True)

    import contextlib

    DR = mybir.MatmulPerfMode.DoubleRow

    with tile.TileContext(nc) as tc:
        with (
            tc.For_i(0, repeat, 1) if repeat > 1 else contextlib.nullcontext(),
            tc.tile_pool(name="persist", bufs=1) as persist,
            tc.tile_pool(name="small", bufs=1) as small,
        ):
            # ---- persistent SBUF tensors ----
            # per head-pair: partitions 0:64 = head 2i, 64:128 = head 2i+1;
            # free dim: [:, 0, :] = Q^T rows, [:, 1, :] = K^T rows
            qk2 = [persist.tile([128, 2, S], bf16, tag=f"qk2_{i}", name=f"qk2_{i}")
                   for i in range(HPC // 2)]
            # V' for DoubleRow: [p, kc2, h, j, 0:64] = VSCALE*V row
            # (sk = kc2*256 + j*128 + p, head h); [..., 64] = VSCALE (ones col
            # -> free softmax denominators). Inner dim padded to 80 so the
            # j-step (80 fp8 bytes) is 16B-aligned for DoubleRow ldweights.
            v65 = persist.tile([128, KC2, HPC, 2, 80], fp8, tag="v65")
            # bf16 V (unscaled, no ones col) for the BV^T matmuls
            v64b = persist.tile([128, SQ, HPC, 64], bf16, tag="v64b")
            y_sb = persist.tile([128, SQ, GD], bf16, tag="y_sb")

            ident_f32 = small.tile([128, 128], fp32, tag="ident_f32")
            make_identity(nc, ident_f32)
            bqk_sb = small.tile([128, 2, 2], fp32, tag="bqk_sb")
            nc.sync.dma_start(out=bqk_sb, in_=bqk[:, :])
            bv_sb = small.tile([1, GD], bf16, tag="bv_sb")
            nc.sync.dma_start(out=bv_sb, in_=bv[:, :])
            ones1 = small.tile([1, 128], bf16, tag="ones1")
            nc.vector.memset(ones1, 1.0)
            nc.vector.memset(v65[:, :, :, :, 64:65], VSCALE)
            eshift = small.tile([128, 1], fp32, tag="eshift")
            nc.vector.memset(eshift, EXP_SHIFT)

            yv = y.rearrange("(c p) n -> p c n", p=128)

            with (
                # phase2a pools: outlive phase 1 (stack allocator is LIFO)
                tc.tile_pool(name="expstp", bufs=2) as expstp,
                tc.tile_pool(name="pspool", bufs=2, space="PSUM") as pspool,
            ):
                def s_chunk(h, e, kc):
                    """S^T chunk kc of head h: 4 matmuls + 2 exp tiles."""
                    pair, off = h // 2, 64 * (h % 2)
                    for t in range(2):
                        ps_s = pspool.tile([128, 1024], fp32, tag="ps",
                                           name="ps_s")
                        for t2 in range(2):
                            c0 = t * 1024 + t2 * 512
                            nc.tensor.matmul(
                                ps_s[:, t2 * 512:(t2 + 1) * 512],
                                qk2[pair][off:off + 64, 1,
                                          kc * 128:(kc + 1) * 128],
                                qk2[pair][off:off + 64, 0, c0:c0 + 512],
                                start=True,
                                stop=True,
                            )
                        nc.scalar.activation(
                            e[:, kc, t * 1024:(t + 1) * 1024],
                            ps_s,
                            mybir.ActivationFunctionType.Exp,
                            bias=eshift[:, 0:1],
                            scale=0.125,
                        )

                def emit_s_interleaved(h, e, fill):
                    """Emit all S^T chunks of head h, draining fill units
                    between chunks to keep the PE FIFO busy."""
                    n = len(fill)
                    done = 0
                    for kc in range(SQ):
                        s_chunk(h, e, kc)
                        upto = (kc + 1) * n // SQ
                        while done < upto:
                            fill[done]()
                            done += 1

                # ---- phase 1 (interleaved with S^T of head 0) ----
                with (
                    tc.tile_pool(name="p1sb", bufs=1) as p1sb,
                    tc.tile_pool(name="p1ps", bufs=2, space="PSUM") as p1ps,
                    tc.tile_pool(name="p1psv", bufs=2, space="PSUM") as p1psv,
                ):
                    wall_sb = p1sb.tile([128, KO, 768], bf16, tag="wall_sb")
                    nc.scalar.dma_start(out=wall_sb, in_=wall[:, :])
                    xts = p1sb.tile([128, KO, S], bf16, tag="xts")
                    xv = xT.rearrange("(ko p) s -> p ko s", p=128)
                    for ko in range(KO):
                        eng = nc.sync if ko % 2 == 0 else nc.scalar
                        eng.dma_start(out=xts[:, ko, :], in_=xv[:, ko, :])

                    def qk_tile(i, qk, t):
                        ps = p1ps.tile([128, 512], fp32, tag="ps_qk",
                                       name="ps_qk")
                        w0 = i * 256 + qk * 128
                        for ko in range(KO):
                            nc.tensor.matmul(
                                ps,
                                wall_sb[:, ko, w0:w0 + 128],
                                xts[:, ko, t * 512:(t + 1) * 512],
                                start=(ko == 0),
                                stop=(ko == KO - 1),
                            )
                        # PSUM -> SBUF + per-partition bias
                        nc.scalar.activation(
                            qk2[i][:, qk, t * 512:(t + 1) * 512],
                            ps,
                            mybir.ActivationFunctionType.Identity,
                            bias=bqk_sb[:, i, qk:qk + 1],
                            scale=1.0,
                        )

                    def v_unit(kc):
                        psv = p1psv.tile([128, GD], fp32, tag="ps_v",
                                         name="ps_v")
                        for ko in range(KO):
                            nc.tensor.matmul(
                                psv,
                                xts[:, ko, kc * 128:(kc + 1) * 128],
                                wall_sb[:, ko, 512:768],
                                start=(ko == 0),
                                stop=False,
                            )
                        nc.tensor.matmul(psv, ones1, bv_sb, start=False,
                                         stop=True)
                        for h in range(HPC):
                            # VSCALE*V, cast to fp8, DoubleRow interleave slot
                            nc.vector.tensor_scalar_mul(
                                v65[:, kc // 2, h, kc % 2, 0:64],
                                psv[:, h * HD:(h + 1) * HD],
                                VSCALE,
                            )
                            nc.vector.tensor_copy(
                                v64b[:, kc, h, :], psv[:, h * HD:(h + 1) * HD]
                            )

                    # head 0 needs pair-0 Q fully and K per chunk: emit pair-0
                    # upfront, interleave the rest with S^T(0)
                    for qk in range(2):
                        for t in range(ST):
                            qk_tile(0, qk, t)
                    fill = []
                    for qk in range(2):
                        for t in range(ST):
                            fill.append(lambda qk=qk, t=t: qk_tile(1, qk, t))
                    for kc in range(SQ):
                        fill.append(lambda kc=kc: v_unit(kc))

                    e_h = [None] * HPC
                    e_h[0] = expstp.tile([128, SQ, S], fp8, tag="expst",
                                         name="expst0")
                    emit_s_interleaved(0, e_h[0], fill)

                # ---- phase 2b: U'/BV + assembly, pipelined with S^T(h) ----
                with (
                    tc.tile_pool(name="ubvpool", bufs=2, space="PSUM") as ubvpool,
                    tc.tile_pool(name="trpool", bufs=2, space="PSUM") as trpool,
                    tc.tile_pool(name="btring", bufs=3) as btring,
                    tc.tile_pool(name="asmpool", bufs=2) as asmpool,
                    tc.tile_pool(name="rcpool", bufs=4) as rcpool,
                ):
                    def ubv_units(h, e):
                        """Work units for U'^T/BV^T + y assembly of head h."""
                        units = []
                        for q in range(4):
                            def load_bt(h=h, q=q):
                                bt = btring.tile([128, SQ, 512], bf16, tag="bt",
                                                 name="bt")
                                eng = nc.sync if q % 2 == 0 else nc.scalar
                                eng.dma_start(out=bt, in_=BT2[h, q])
                                return bt

                            # closure cell: bt tile created lazily at drain time
                            cell = {}

                            def start_q(cell=cell, load_bt=load_bt, h=h, q=q):
                                cell["bt"] = load_bt()
                                cell["u"] = ubvpool.tile([65, 512], fp32,
                                                         tag="ub", name="u")
                                cell["b"] = ubvpool.tile([64, 512], fp32,
                                                         tag="ub", name="bb")

                            units.append(start_q)

                            def mm_pair(kc2, cell=cell, h=h, q=q, e=e):
                                st, sp = (kc2 == 0), (kc2 == KC2 - 1)
                                nc.tensor.matmul(
                                    cell["u"],
                                    v65[:, kc2, h, :, 0:65],
                                    e[:, 2 * kc2:2 * kc2 + 2,
                                      q * 512:(q + 1) * 512],
                                    start=st, stop=sp, perf_mode=DR,
                                )
                                for j in range(2):
                                    kc = 2 * kc2 + j
                                    nc.tensor.matmul(
                                        cell["b"],
                                        v64b[:, kc, h, :],
                                        cell["bt"][:, kc, :],
                                        start=(kc == 0), stop=(kc == SQ - 1),
                                    )

                            for kc2 in range(KC2):
                                units.append(lambda kc2=kc2, f=mm_pair: f(kc2))

                            def copies(cell=cell):
                                usb = asmpool.tile([65, 512], fp32, tag="usb",
                                                   name="usb")
                                bsb = asmpool.tile([64, 512], fp32, tag="bsb",
                                                   name="bsb")
                                nc.vector.tensor_copy(usb, cell["u"])
                                nc.vector.tensor_copy(bsb, cell["b"])
                                cell["usb"], cell["bsb"] = usb, bsb

                            units.append(copies)

                            def asm(cc, cell=cell, h=h, q=q):
                                c = q * 4 + cc
                                tr = trpool.tile([128, 2, 65], fp32, tag="tr",
                                                 name="tr")
                                nc.tensor.transpose(
                                    tr[:, 0, :],
                                    cell["usb"][:, cc * 128:(cc + 1) * 128],
                                    ident_f32[0:65, 0:65],
                                )
                                nc.tensor.transpose(
                                    tr[:, 1, 0:64],
                                    cell["bsb"][:, cc * 128:(cc + 1) * 128],
                                    ident_f32[0:64, 0:64],
                                )
                                rc = rcpool.tile([128, 1], fp32, tag="rc",
                                                 name="rc")
                                nc.vector.reciprocal(rc, tr[:, 0, 64:65])
                                # y = U*recip + BV (two ops: only one PSUM
                                # input allowed per DVE instruction)
                                ys = y_sb[:, c, h * HD:(h + 1) * HD]
                                nc.vector.tensor_scalar_mul(
                                    ys, tr[:, 0, 0:64], rc
                                )
                                nc.vector.tensor_tensor(
                                    ys, ys, tr[:, 1, 0:64],
                                    mybir.AluOpType.add,
                                )
                                if h == HPC - 1:
                                    nc.sync.dma_start(
                                        out=yv[:, c, :], in_=y_sb[:, c, :]
                                    )

                            for cc in range(4):
                                units.append(lambda cc=cc, f=asm: f(cc))
                        return units

                    for h in range(1, HPC):
                        e_h[h] = expstp.tile([128, SQ, S], fp8, tag="expst",
                                             name=f"expst{h}")
                        emit_s_interleaved(h, e_h[h], ubv_units(h - 1, e_h[h - 1]))
                    for unit in ubv_units(HPC - 1, e_h[HPC - 1]):
                        unit()

    nc.finalize()
    return nc


def _prep_core_inputs(x, attn_B, W_attn, b_attn, core, BT_all=None):
    bi, g = core // 4, core % 4
    h0 = HPC * g
    xT = _to_bf16(np.ascontiguousarray(x[bi].T))             # [D, S]
    wqk = np.empty((D, 2, 2, 128), np.float32)
    bqk = np.empty((128, 2, 2), np.float32)
    for i in range(HPC // 2):
        for j in range(2):                                   # head within pair
            gh = h0 + 2 * i + j
            sl = slice(64 * j, 64 * j + 64)
            wqk[:, i, 0, sl] = W_attn[:, gh * 64:(gh + 1) * 64]
            wqk[:, i, 1, sl] = W_attn[:, D + gh * 64:D + (gh + 1) * 64]
            bqk[sl, i, 0] = b_attn[gh * 64:(gh + 1) * 64]
            bqk[sl, i, 1] = b_attn[D + gh * 64:D + (gh + 1) * 64]
    wv = W_attn[:, 2 * D + g * GD:2 * D + (g + 1) * GD]
    bv = b_attn[2 * D + g * GD:2 * D + (g + 1) * GD]
    # pack per-partition-contiguous: wall[p, ko, :512] = wqk, [512:] = wv
    wall = np.empty((128, KO, 768), np.float32)
    wall[:, :, 0:512] = wqk.reshape(KO, 128, 512).transpose(1, 0, 2)
    wall[:, :, 512:768] = wv.reshape(KO, 128, GD).transpose(1, 0, 2)
    if BT_all is None:
        BT_all = _prep_bt(attn_B)
    BT2 = np.ascontiguousarray(BT_all[bi, h0:h0 + HPC])
    return {
        "xT": xT, "wall": _to_bf16(wall), "bqk": bqk,
        "bv": _to_bf16(bv.reshape(1, GD)), "BT2": BT2,
    }


def _prep_bt(attn_B):
    """[b, h, sq, sk] f32 -> [b, h, q, p, kc, c] bf16 where
    BT2[b, h, q, p, kc, c] = attn_B[b, h, q*512+c, kc*128+p]."""
    a = _to_bf16(attn_B)                           # [b, h, 2048(sq), 2048(sk)]
    a = a.reshape(B, H, 4, 512, SQ, 128)           # [b, h, q, c, kc, p]
    return a.transpose(0, 1, 2, 5, 4, 3)           # [b, h, q, p, kc, c]


def kernel(x, attn_B, W_attn, b_attn):
    global _CACHED_NC
    x = np.asarray(x, np.float32)
    attn_B = np.asarray(attn_B, np.float32)
    W_attn = np.asarray(W_attn, np.float32)
    b_attn = np.asarray(b_attn, np.float32)

    if _CACHED_NC is None:
        _CACHED_NC = build_nc()
    nc = _CACHED_NC

    BT_all = np.ascontiguousarray(_prep_bt(attn_B))
    in_maps = [
        _prep_core_inputs(x, attn_B, W_attn, b_attn, c, BT_all=BT_all)
        for c in range(NCORES)
    ]
    res = run_bass_kernel_spmd(nc, in_maps, core_ids=list(range(NCORES)))

    out = np.empty((B, S, D), np.float32)
    for c in range(NCORES):
        bi, g = c // 4, c % 4
        out[bi, :, g * GD:(g + 1) * GD] = res.results[c]["y"].astype(np.float32)
    return out
